# revision 32
# baseline (speedup 1.0000x reference)
"""Multi-head attention Trainium2 Bass kernel.

Problem: x:(4,512,1024), Wq/Wk/Wv/Wo:(512,512), H=8 heads, d=64.
  q = Wq@x ; k = Wk@x ; v = Wv@x  (per batch, 1x1 conv == channel matmul)
  per head: S[i,j] = q[:,i].k[:,j] ; attn = softmax_j(S) ; y = attn @ v
  out = Wo @ y

Sharding: 8 cores = (batch b, head-half g).  Core (b,g) handles batch b,
local heads g*4..g*4+3 and computes the partial output
out_p = Wo[:, g*256:(g+1)*256] @ y_g which the host sums pairwise.

Layout: scores are computed TRANSPOSED (S^T[j,i] = k^T q) so no PE
transposes are needed anywhere; softmax runs without max subtraction
(scores max ~52 < 88 overflow limit); the PV matmul's lhsT carries an
extra ones column so the softmax denominator falls out of the same
matmul; the rowsum row is replicated across partitions by a K=1
ones-row matmul and inverted with a fast approximate reciprocal.

All matmuls are float32r: 1 row/cycle at 2.4 GHz once the PE clock gate
(HAM) is warm, with fp32-level dynamic range and ~1e-3 accuracy.
"""

import numpy as np

import concourse.bass as bass
import concourse.tile as tile
from concourse import bacc
from concourse import mybir
from concourse.bass_utils import run_bass_kernel_spmd

F32 = mybir.dt.float32
F32R = mybir.dt.float32r
BF16 = mybir.dt.bfloat16

P = 128
C = 512          # channels
NSEQ = 1024      # sequence length
D = 64           # head dim
HL = 4           # local heads per core
KC = C // P      # 4 contraction tiles over channels
J = NSEQ // P    # 8 key tiles

_NC_CACHE = {}


def build_nc():
    nc = bacc.Bacc("TRN2")

    x = nc.dram_tensor("x", [C, NSEQ], F32R, kind="ExternalInput")
    wqkv = nc.dram_tensor("wqkv_t", [C, 3, 2 * P], F32R, kind="ExternalInput")
    wo = nc.dram_tensor("wo_t", [D, HL, C], F32R, kind="ExternalInput")
    out = nc.dram_tensor("out_p", [C, NSEQ], F32, kind="ExternalOutput")

    with tile.TileContext(nc) as tc:
        with (
            tc.tile_pool(name="consts", bufs=1) as consts,
            tc.tile_pool(name="epool", bufs=6) as epool,
            tc.tile_pool(name="ypool", bufs=6) as ypool,
            tc.tile_pool(name="rpool", bufs=4) as rpool,
            tc.tile_pool(name="opool", bufs=2) as opool,
            tc.tile_pool(name="pp", bufs=2, space="PSUM") as pp,
        ):
            # ---- load inputs: per-k-tile tiles so Tile's per-tile dep
            # tracking lets each projection matmul start as soon as ITS
            # chunk is in, not after the whole load.
            x_t = x.rearrange("(kc p) n -> p kc n", p=P)
            w_t = wqkv.rearrange("(kc p) w m -> p kc w m", p=P)
            x_sb, w3_sb = [], []
            for kc in range(KC):
                t = consts.tile([P, 3, 2 * P], F32R, tag=f"w{kc}")
                nc.sync.dma_start(t, w_t[:, kc])
                w3_sb.append(t)
                xt = consts.tile([P, NSEQ], F32R, tag=f"x{kc}")
                nc.sync.dma_start(xt, x_t[:, kc])
                x_sb.append(xt)
            wq_sb = [t[:, 0, :] for t in w3_sb]
            wk_sb = [t[:, 1, :] for t in w3_sb]
            wv_sb = [t[:, 2, :] for t in w3_sb]
            # dependency-free warm-up matmuls fill the PE during the load
            # window so the HAM clock gate reaches 8/8 before projections.
            warm_sb = consts.tile([P, 512], F32R)
            nc.vector.memset(warm_sb.bitcast(mybir.dt.uint32), 0)
            for wi in range(30):
                pw = pp.tile([P, 512], F32, tag="po", name="pw", bufs=2)
                nc.tensor.matmul(pw, lhsT=warm_sb[:, 0:P], rhs=warm_sb,
                                 start=True, stop=True)
            wot_sb = consts.tile([D, HL, C], F32R)
            nc.sync.dma_start(wot_sb, wo[:, :, :])

            # ---- q, k projections: (256,512)@(512,1024) ----
            q_sb = [consts.tile([P, NSEQ], F32R, tag=f"q{m}", name=f"q{m}")
                    for m in range(2)]
            k_sb = [consts.tile([P, NSEQ], F32R, tag=f"k{m}", name=f"k{m}")
                    for m in range(2)]
            tags = ["po", "py0", "py1"]
            ti = 0
            for w_sb, dst in ((wq_sb, q_sb), (wk_sb, k_sb)):
                for m in range(2):
                    for nn in range(2):
                        tag = tags[ti % 3]; ti += 1
                        ps = pp.tile([P, 512], F32, tag=tag, name=tag,
                                     bufs=2 if tag == "po" else 1)
                        for kc in range(KC):
                            nc.tensor.matmul(
                                ps,
                                lhsT=w_sb[kc][:, m * P:(m + 1) * P],
                                rhs=x_sb[kc][:, nn * 512:(nn + 1) * 512],
                                start=(kc == 0),
                                stop=(kc == KC - 1),
                            )
                        nc.vector.tensor_copy(
                            out=dst[m][:, nn * 512:(nn + 1) * 512], in_=ps
                        )

            # ---- v^T projection: out[j, d'] tiles, plus ones column ----
            vt_sb = [
                consts.tile([P, HL, D + 1], F32R, tag=f"vt{j}", name=f"vt{j}")
                for j in range(J)
            ]
            # memset can't target f32r; zero the ones-column via a uint32
            # view, then produce rounded-f32r 1.0s with ACT (0*x + 1).
            for j in range(J):
                ones_col = vt_sb[j][:, :, D:D + 1]
                nc.vector.memset(ones_col.bitcast(mybir.dt.uint32), 0)
                nc.scalar.activation(
                    out=ones_col, in_=ones_col,
                    func=mybir.ActivationFunctionType.Identity,
                    bias=1.0, scale=0.0,
                )
            for j in range(J):
                tag = tags[ti % 3]; ti += 1
                psv = pp.tile([P, 512], F32, tag=tag, name=tag,
                              bufs=2 if tag == "po" else 1)
                for kc in range(KC):
                    nc.tensor.matmul(
                        psv[:, 0:2 * P],
                        lhsT=x_sb[kc][:, j * P:(j + 1) * P],
                        rhs=wv_sb[kc],
                        start=(kc == 0),
                        stop=(kc == KC - 1),
                    )
                nc.vector.tensor_copy(
                    out=vt_sb[j][:, :, 0:D],
                    in_=psv[:, 0:2 * P].rearrange("p (h d) -> p h d", h=HL),
                )

            ones64 = consts.tile([P, D], F32R)
            nc.vector.memset(ones64.bitcast(mybir.dt.uint32), 0)
            nc.scalar.activation(
                out=ones64[D:D + 1, :], in_=ones64[D:D + 1, :],
                func=mybir.ActivationFunctionType.Identity,
                bias=1.0, scale=0.0,
            )

            # ---- attention: head pairs on alternating PE row groups,
            # i processed in halves; outproj per i-half overlaps the next
            # half's attention.
            y_sb = [
                consts.tile([D, NSEQ], F32R, tag=f"y{h}", name=f"y{h}")
                for h in range(HL)
            ]
            out_t = out.rearrange("(m p) n -> p m n", p=P)
            pending = []

            def emit_pending():
                for fn in pending:
                    fn()
                pending.clear()

            for ihalf in range(2):
                isl = slice(ihalf * 512, (ihalf + 1) * 512)
                for pair in ((0, 1) if ihalf == 0 else (1, 0)):
                    py = [
                        pp.tile([P, 512], F32, tag="py0", name="py0", bufs=1),
                        pp.tile([P, 512], F32, tag="py1", name="py1", bufs=1),
                    ]
                    for j in range(J):
                        # both lanes' scores into one 2-bank psum tile so a
                        # single 1024-wide exp serves the head pair
                        ps = pp.tile([P, 1024], F32, tag="s", name="s",
                                     bufs=2)
                        for lane in range(2):
                            hp = lane * D
                            nc.tensor.matmul(
                                ps[:, lane * 512:(lane + 1) * 512],
                                lhsT=k_sb[pair][hp:hp + D, j * P:(j + 1) * P],
                                rhs=q_sb[pair][hp:hp + D, isl],
                                start=True, stop=True,
                            )
                        e = epool.tile([P, 1024], F32R, tag="e", name="e")
                        nc.scalar.activation(
                            out=e, in_=ps,
                            func=mybir.ActivationFunctionType.Exp,
                        )
                        for lane in range(2):
                            nc.tensor.matmul(
                                py[lane][0:D + 1, :],
                                lhsT=vt_sb[j][:, 2 * pair + lane, :],
                                rhs=e[:, lane * 512:(lane + 1) * 512],
                                start=(j == 0), stop=(j == J - 1),
                            )
                    # the 65-row copies release the PSUM accumulators now;
                    # the replicate/1-over-x/scale tail is emitted one block
                    # later so its PE work lands in scheduling slack instead
                    # of pinching the next block's first scores.
                    emit_pending()
                    yus = []
                    for lane in range(2):
                        yu = ypool.tile([D + 1, 512], F32R, tag="yu",
                                        name="yu")
                        nc.vector.tensor_copy(out=yu, in_=py[lane][0:D + 1, :])
                        yus.append(yu)

                    def drain(yus=yus, pair=pair, isl=isl):
                        for lane in range(2):
                            h = 2 * pair + lane
                            pr = pp.tile([D, 512], F32, tag=f"py{lane}",
                                         name="pr", bufs=1)
                            nc.tensor.matmul(
                                pr, lhsT=ones64[D:D + 1, :],
                                rhs=yus[lane][D:D + 1, :],
                                start=True, stop=True,
                            )
                            rr = rpool.tile([D, 512], F32, tag="rr",
                                            name="rr")
                            nc.vector.reciprocal_approx_fast(out=rr, in_=pr)
                            nc.vector.tensor_tensor(
                                out=y_sb[h][:, isl],
                                in0=yus[lane][0:D, :], in1=rr,
                                op=mybir.AluOpType.mult,
                            )

                    pending.append(drain)
                emit_pending()
                # output projection for this i-half (hoists into the next
                # half's attention once the 4 y tiles are ready)
                horder = (0, 1, 2, 3) if ihalf == 0 else (2, 3, 0, 1)
                for m in range(4):
                    po = pp.tile([P, 512], F32, tag="po", name="po", bufs=2)
                    for hi, h in enumerate(horder):
                        nc.tensor.matmul(
                            po,
                            lhsT=wot_sb[:, h, m * P:(m + 1) * P],
                            rhs=y_sb[h][:, isl],
                            start=(hi == 0),
                            stop=(hi == HL - 1),
                        )
                    ot = opool.tile([P, 512], F32, tag="ot")
                    nc.vector.tensor_copy(out=ot, in_=po)
                    nc.sync.dma_start(out=out_t[:, m, isl], in_=ot)

    nc.compile()
    return nc


def get_nc():
    if "nc" not in _NC_CACHE:
        _NC_CACHE["nc"] = build_nc()
    return _NC_CACHE["nc"]


def make_in_maps(x, Wq, Wk, Wv, Wo):
    in_maps = []
    for core in range(8):
        b, g = core // 2, core % 2
        sl = slice(g * 256, (g + 1) * 256)
        wqkv = np.stack(
            [Wq[sl, :].T, Wk[sl, :].T, Wv[sl, :].T], axis=1
        )  # (512, 3, 256)
        in_maps.append({
            "x": np.ascontiguousarray(x[b]),
            "wqkv_t": np.ascontiguousarray(wqkv),
            # [d, h, o] so lhsT slices are contiguous per head
            "wo_t": np.ascontiguousarray(
                Wo[:, sl].reshape(C, HL, D).transpose(2, 1, 0)
            ),
        })
    return in_maps


LAST_RESULTS = {}


def kernel(x, Wq, Wk, Wv, Wo, _trace=False):
    x = np.asarray(x, dtype=np.float32)
    Wq = np.asarray(Wq, dtype=np.float32)
    Wk = np.asarray(Wk, dtype=np.float32)
    Wv = np.asarray(Wv, dtype=np.float32)
    Wo = np.asarray(Wo, dtype=np.float32)

    nc = get_nc()
    in_maps = make_in_maps(x, Wq, Wk, Wv, Wo)
    res = run_bass_kernel_spmd(
        nc, in_maps, core_ids=list(range(8)), trace=_trace
    )
    LAST_RESULTS["res"] = res
    parts = [np.asarray(r["out_p"]) for r in res.results]
    out = np.stack([parts[2 * b] + parts[2 * b + 1] for b in range(4)])
    return out


# revision 33
# speedup vs baseline: 1.0025x; 1.0025x over previous
"""Multi-head attention Trainium2 Bass kernel.

Problem: x:(4,512,1024), Wq/Wk/Wv/Wo:(512,512), H=8 heads, d=64.
  q = Wq@x ; k = Wk@x ; v = Wv@x  (per batch, 1x1 conv == channel matmul)
  per head: S[i,j] = q[:,i].k[:,j] ; attn = softmax_j(S) ; y = attn @ v
  out = Wo @ y

Sharding: 8 cores = (batch b, head-half g).  Core (b,g) handles batch b,
local heads g*4..g*4+3 and computes the partial output
out_p = Wo[:, g*256:(g+1)*256] @ y_g which the host sums pairwise.

Layout: scores are computed TRANSPOSED (S^T[j,i] = k^T q) so no PE
transposes are needed anywhere; softmax runs without max subtraction
(scores max ~52 < 88 overflow limit); the PV matmul's lhsT carries an
extra ones column so the softmax denominator falls out of the same
matmul; the rowsum row is replicated across partitions by a K=1
ones-row matmul and inverted with a fast approximate reciprocal.

All matmuls are float32r: 1 row/cycle at 2.4 GHz once the PE clock gate
(HAM) is warm, with fp32-level dynamic range and ~1e-3 accuracy.
"""

import numpy as np

import concourse.bass as bass
import concourse.tile as tile
from concourse import bacc
from concourse import mybir
from concourse.bass_utils import run_bass_kernel_spmd

F32 = mybir.dt.float32
F32R = mybir.dt.float32r
BF16 = mybir.dt.bfloat16

P = 128
C = 512          # channels
NSEQ = 1024      # sequence length
D = 64           # head dim
HL = 4           # local heads per core
KC = C // P      # 4 contraction tiles over channels
J = NSEQ // P    # 8 key tiles

_NC_CACHE = {}


def build_nc():
    nc = bacc.Bacc("TRN2")

    x = nc.dram_tensor("x", [C, NSEQ], F32R, kind="ExternalInput")
    wqkv = nc.dram_tensor("wqkv_t", [C, 3, 2 * P], F32R, kind="ExternalInput")
    wo = nc.dram_tensor("wo_t", [D, HL, C], F32R, kind="ExternalInput")
    out = nc.dram_tensor("out_p", [C, NSEQ], F32, kind="ExternalOutput")

    with tile.TileContext(nc) as tc:
        with (
            tc.tile_pool(name="consts", bufs=1) as consts,
            tc.tile_pool(name="epool", bufs=6) as epool,
            tc.tile_pool(name="ypool", bufs=6) as ypool,
            tc.tile_pool(name="rpool", bufs=4) as rpool,
            tc.tile_pool(name="opool", bufs=2) as opool,
            tc.tile_pool(name="pp", bufs=2, space="PSUM") as pp,
        ):
            # ---- load inputs: per-k-tile tiles so Tile's per-tile dep
            # tracking lets each projection matmul start as soon as ITS
            # chunk is in, not after the whole load.
            x_t = x.rearrange("(kc p) n -> p kc n", p=P)
            w_t = wqkv.rearrange("(kc p) w m -> p kc w m", p=P)
            x_sb, w3_sb = [], []
            for kc in range(KC):
                t = consts.tile([P, 3, 2 * P], F32R, tag=f"w{kc}")
                nc.sync.dma_start(t, w_t[:, kc])
                w3_sb.append(t)
                xt = consts.tile([P, NSEQ], F32R, tag=f"x{kc}")
                nc.sync.dma_start(xt, x_t[:, kc])
                x_sb.append(xt)
            wq_sb = [t[:, 0, :] for t in w3_sb]
            wk_sb = [t[:, 1, :] for t in w3_sb]
            wv_sb = [t[:, 2, :] for t in w3_sb]
            # dependency-free warm-up matmuls fill the PE during the load
            # window so the HAM clock gate reaches 8/8 before projections.
            warm_sb = consts.tile([P, 512], F32R)
            nc.vector.memset(warm_sb.bitcast(mybir.dt.uint32), 0)
            for wi in range(30):
                pw = pp.tile([P, 512], F32, tag="po", name="pw", bufs=2)
                nc.tensor.matmul(pw, lhsT=warm_sb[:, 0:P], rhs=warm_sb,
                                 start=True, stop=True)
            wot_sb = consts.tile([D, HL, C], F32R)
            nc.sync.dma_start(wot_sb, wo[:, :, :])

            # ---- q, k projections: (256,512)@(512,1024) ----
            q_sb = [consts.tile([P, NSEQ], F32R, tag=f"q{m}", name=f"q{m}")
                    for m in range(2)]
            k_sb = [consts.tile([P, NSEQ], F32R, tag=f"k{m}", name=f"k{m}")
                    for m in range(2)]
            tags = ["po", "py0", "py1"]
            ti = 0
            for w_sb, dst in ((wq_sb, q_sb), (wk_sb, k_sb)):
                for m in range(2):
                    for nn in range(2):
                        tag = tags[ti % 3]; ti += 1
                        ps = pp.tile([P, 512], F32, tag=tag, name=tag,
                                     bufs=2 if tag == "po" else 1)
                        for kc in range(KC):
                            nc.tensor.matmul(
                                ps,
                                lhsT=w_sb[kc][:, m * P:(m + 1) * P],
                                rhs=x_sb[kc][:, nn * 512:(nn + 1) * 512],
                                start=(kc == 0),
                                stop=(kc == KC - 1),
                            )
                        nc.vector.tensor_copy(
                            out=dst[m][:, nn * 512:(nn + 1) * 512], in_=ps
                        )

            # ---- v^T projection: out[j, d'] tiles, plus ones column ----
            vt_sb = [
                consts.tile([P, HL, D + 1], F32R, tag=f"vt{j}", name=f"vt{j}")
                for j in range(J)
            ]
            # memset can't target f32r; zero the ones-column via a uint32
            # view, then produce rounded-f32r 1.0s with ACT (0*x + 1).
            for j in range(J):
                ones_col = vt_sb[j][:, :, D:D + 1]
                nc.vector.memset(ones_col.bitcast(mybir.dt.uint32), 0)
                nc.scalar.activation(
                    out=ones_col, in_=ones_col,
                    func=mybir.ActivationFunctionType.Identity,
                    bias=1.0, scale=0.0,
                )
            for j in range(J):
                tag = tags[ti % 3]; ti += 1
                psv = pp.tile([P, 512], F32, tag=tag, name=tag,
                              bufs=2 if tag == "po" else 1)
                for kc in range(KC):
                    nc.tensor.matmul(
                        psv[:, 0:2 * P],
                        lhsT=x_sb[kc][:, j * P:(j + 1) * P],
                        rhs=wv_sb[kc],
                        start=(kc == 0),
                        stop=(kc == KC - 1),
                    )
                nc.vector.tensor_copy(
                    out=vt_sb[j][:, :, 0:D],
                    in_=psv[:, 0:2 * P].rearrange("p (h d) -> p h d", h=HL),
                )

            ones64 = consts.tile([P, D], F32R)
            nc.vector.memset(ones64.bitcast(mybir.dt.uint32), 0)
            nc.scalar.activation(
                out=ones64[D:D + 1, :], in_=ones64[D:D + 1, :],
                func=mybir.ActivationFunctionType.Identity,
                bias=1.0, scale=0.0,
            )

            # ---- attention: head pairs on alternating PE row groups,
            # i processed in halves; outproj per i-half overlaps the next
            # half's attention.
            y_sb = [
                consts.tile([D, NSEQ], F32R, tag=f"y{h}", name=f"y{h}")
                for h in range(HL)
            ]
            out_t = out.rearrange("(m p) n -> p m n", p=P)
            pending = []

            def emit_pending():
                for fn in pending:
                    fn()
                pending.clear()

            for ihalf in range(2):
                isl = slice(ihalf * 512, (ihalf + 1) * 512)
                for pair in ((0, 1) if ihalf == 0 else (1, 0)):
                    py = [
                        pp.tile([P, 512], F32, tag="py0", name="py0", bufs=1),
                        pp.tile([P, 512], F32, tag="py1", name="py1", bufs=1),
                    ]
                    for j in range(J):
                        # both lanes' scores into one 2-bank psum tile so a
                        # single 1024-wide exp serves the head pair
                        ps = pp.tile([P, 1024], F32, tag="s", name="s",
                                     bufs=2)
                        for lane in range(2):
                            hp = lane * D
                            nc.tensor.matmul(
                                ps[:, lane * 512:(lane + 1) * 512],
                                lhsT=k_sb[pair][hp:hp + D, j * P:(j + 1) * P],
                                rhs=q_sb[pair][hp:hp + D, isl],
                                start=True, stop=True,
                            )
                        e = epool.tile([P, 1024], F32R, tag="e", name="e")
                        nc.scalar.activation(
                            out=e, in_=ps,
                            func=mybir.ActivationFunctionType.Exp,
                        )
                        for lane in range(2):
                            nc.tensor.matmul(
                                py[lane][0:D + 1, :],
                                lhsT=vt_sb[j][:, 2 * pair + lane, :],
                                rhs=e[:, lane * 512:(lane + 1) * 512],
                                start=(j == 0), stop=(j == J - 1),
                            )
                    # the 65-row copies release the PSUM accumulators now;
                    # the replicate/1-over-x/scale tail is emitted one block
                    # later so its PE work lands in scheduling slack instead
                    # of pinching the next block's first scores.
                    emit_pending()
                    yus = []
                    for lane in range(2):
                        yu = ypool.tile([D + 1, 512], F32R, tag="yu",
                                        name="yu")
                        nc.vector.tensor_copy(out=yu, in_=py[lane][0:D + 1, :])
                        yus.append(yu)

                    def drain(yus=yus, pair=pair, isl=isl):
                        for lane in range(2):
                            h = 2 * pair + lane
                            pr = pp.tile([D, 512], F32, tag=f"py{lane}",
                                         name="pr", bufs=1)
                            nc.tensor.matmul(
                                pr, lhsT=ones64[D:D + 1, :],
                                rhs=yus[lane][D:D + 1, :],
                                start=True, stop=True,
                            )
                            rr = rpool.tile([D, 512], F32, tag="rr",
                                            name="rr")
                            nc.vector.reciprocal_approx_fast(out=rr, in_=pr)
                            nc.vector.tensor_tensor(
                                out=y_sb[h][:, isl],
                                in0=yus[lane][0:D, :], in1=rr,
                                op=mybir.AluOpType.mult,
                            )

                    drain()
                emit_pending()
                # output projection for this i-half (hoists into the next
                # half's attention once the 4 y tiles are ready)
                horder = (0, 1, 2, 3) if ihalf == 0 else (2, 3, 0, 1)
                for m in range(4):
                    po = pp.tile([P, 512], F32, tag="po", name="po", bufs=2)
                    for hi, h in enumerate(horder):
                        nc.tensor.matmul(
                            po,
                            lhsT=wot_sb[:, h, m * P:(m + 1) * P],
                            rhs=y_sb[h][:, isl],
                            start=(hi == 0),
                            stop=(hi == HL - 1),
                        )
                    ot = opool.tile([P, 512], F32, tag="ot")
                    nc.vector.tensor_copy(out=ot, in_=po)
                    nc.sync.dma_start(out=out_t[:, m, isl], in_=ot)

    nc.compile()
    return nc


def get_nc():
    if "nc" not in _NC_CACHE:
        _NC_CACHE["nc"] = build_nc()
    return _NC_CACHE["nc"]


def make_in_maps(x, Wq, Wk, Wv, Wo):
    in_maps = []
    for core in range(8):
        b, g = core // 2, core % 2
        sl = slice(g * 256, (g + 1) * 256)
        wqkv = np.stack(
            [Wq[sl, :].T, Wk[sl, :].T, Wv[sl, :].T], axis=1
        )  # (512, 3, 256)
        in_maps.append({
            "x": np.ascontiguousarray(x[b]),
            "wqkv_t": np.ascontiguousarray(wqkv),
            # [d, h, o] so lhsT slices are contiguous per head
            "wo_t": np.ascontiguousarray(
                Wo[:, sl].reshape(C, HL, D).transpose(2, 1, 0)
            ),
        })
    return in_maps


LAST_RESULTS = {}


def kernel(x, Wq, Wk, Wv, Wo, _trace=False):
    x = np.asarray(x, dtype=np.float32)
    Wq = np.asarray(Wq, dtype=np.float32)
    Wk = np.asarray(Wk, dtype=np.float32)
    Wv = np.asarray(Wv, dtype=np.float32)
    Wo = np.asarray(Wo, dtype=np.float32)

    nc = get_nc()
    in_maps = make_in_maps(x, Wq, Wk, Wv, Wo)
    res = run_bass_kernel_spmd(
        nc, in_maps, core_ids=list(range(8)), trace=_trace
    )
    LAST_RESULTS["res"] = res
    parts = [np.asarray(r["out_p"]) for r in res.results]
    out = np.stack([parts[2 * b] + parts[2 * b + 1] for b in range(4)])
    return out


# revision 34
# speedup vs baseline: 1.0944x; 1.0917x over previous
"""Multi-head attention Trainium2 Bass kernel.

Problem: x:(4,512,1024), Wq/Wk/Wv/Wo:(512,512), H=8 heads, d=64.
  q = Wq@x ; k = Wk@x ; v = Wv@x  (per batch, 1x1 conv == channel matmul)
  per head: S[i,j] = q[:,i].k[:,j] ; attn = softmax_j(S) ; y = attn @ v
  out = Wo @ y

Sharding: 8 cores = (batch b, head-half g).  Core (b,g) handles batch b,
local heads g*4..g*4+3 and computes the partial output
out_p = Wo[:, g*256:(g+1)*256] @ y_g which the host sums pairwise.

Layout: scores are computed TRANSPOSED (S^T[j,i] = k^T q) so no PE
transposes are needed anywhere; softmax runs without max subtraction
(scores max ~52 < 88 overflow limit); the PV matmul's lhsT carries an
extra ones column so the softmax denominator falls out of the same
matmul; the rowsum row is replicated across partitions by a K=1
ones-row matmul and inverted with a fast approximate reciprocal.

All matmuls are float32r: 1 row/cycle at 2.4 GHz once the PE clock gate
(HAM) is warm, with fp32-level dynamic range and ~1e-3 accuracy.
"""

import numpy as np

import concourse.bass as bass
import concourse.tile as tile
from concourse import bacc
from concourse import mybir
from concourse.bass_utils import run_bass_kernel_spmd

F32 = mybir.dt.float32
F32R = mybir.dt.float32r
BF16 = mybir.dt.bfloat16

P = 128
C = 512          # channels
NSEQ = 1024      # sequence length
D = 64           # head dim
HL = 4           # local heads per core
KC = C // P      # 4 contraction tiles over channels
J = NSEQ // P    # 8 key tiles

_NC_CACHE = {}


def build_nc():
    nc = bacc.Bacc("TRN2")

    x = nc.dram_tensor("x", [C, NSEQ], F32R, kind="ExternalInput")
    wqkv = nc.dram_tensor("wqkv_t", [C, 3, 2 * P], F32R, kind="ExternalInput")
    wo = nc.dram_tensor("wo_t", [D, HL, C], F32R, kind="ExternalInput")
    out = nc.dram_tensor("out_p", [C, NSEQ], F32, kind="ExternalOutput")

    with tile.TileContext(nc) as tc:
        with (
            tc.tile_pool(name="consts", bufs=1) as consts,
            tc.tile_pool(name="epool", bufs=6) as epool,
            tc.tile_pool(name="ypool", bufs=6) as ypool,
            tc.tile_pool(name="rpool", bufs=4) as rpool,
            tc.tile_pool(name="opool", bufs=2) as opool,
            tc.tile_pool(name="pp", bufs=2, space="PSUM") as pp,
        ):
            # ---- load inputs: per-k-tile tiles so Tile's per-tile dep
            # tracking lets each projection matmul start as soon as ITS
            # chunk is in, not after the whole load.
            x_t = x.rearrange("(kc p) n -> p kc n", p=P)
            w_t = wqkv.rearrange("(kc p) w m -> p kc w m", p=P)
            x_sb, w3_sb = [], []
            for kc in range(KC):
                t = consts.tile([P, 3, 2 * P], F32R, tag=f"w{kc}")
                nc.sync.dma_start(t, w_t[:, kc])
                w3_sb.append(t)
                xt = consts.tile([P, NSEQ], F32R, tag=f"x{kc}")
                nc.sync.dma_start(xt, x_t[:, kc])
                x_sb.append(xt)
            wq_sb = [t[:, 0, :] for t in w3_sb]
            wk_sb = [t[:, 1, :] for t in w3_sb]
            wv_sb = [t[:, 2, :] for t in w3_sb]
            # dependency-free warm-up matmuls fill the PE during the load
            # window so the HAM clock gate reaches 8/8 before projections.
            warm_sb = consts.tile([P, 512], F32R)
            nc.vector.memset(warm_sb.bitcast(mybir.dt.uint32), 0)
            for wi in range(10):
                pw = pp.tile([P, 512], F32, tag="po", name="pw", bufs=2)
                nc.tensor.matmul(pw, lhsT=warm_sb[:, 0:P], rhs=warm_sb,
                                 start=True, stop=True)
            wot_sb = consts.tile([D, HL, C], F32R)
            nc.sync.dma_start(wot_sb, wo[:, :, :])

            # ---- q, k projections: (256,512)@(512,1024) ----
            q_sb = [consts.tile([P, NSEQ], F32R, tag=f"q{m}", name=f"q{m}")
                    for m in range(2)]
            k_sb = [consts.tile([P, NSEQ], F32R, tag=f"k{m}", name=f"k{m}")
                    for m in range(2)]
            tags = ["po", "py0", "py1"]
            ti = 0
            for w_sb, dst in ((wq_sb, q_sb), (wk_sb, k_sb)):
                for m in range(2):
                    for nn in range(2):
                        tag = tags[ti % 3]; ti += 1
                        ps = pp.tile([P, 512], F32, tag=tag, name=tag,
                                     bufs=2 if tag == "po" else 1)
                        for kc in range(KC):
                            nc.tensor.matmul(
                                ps,
                                lhsT=w_sb[kc][:, m * P:(m + 1) * P],
                                rhs=x_sb[kc][:, nn * 512:(nn + 1) * 512],
                                start=(kc == 0),
                                stop=(kc == KC - 1),
                            )
                        nc.vector.tensor_copy(
                            out=dst[m][:, nn * 512:(nn + 1) * 512], in_=ps
                        )

            # ---- v^T projection: out[j, d'] tiles, plus ones column ----
            vt_sb = [
                consts.tile([P, HL, D + 1], F32R, tag=f"vt{j}", name=f"vt{j}")
                for j in range(J)
            ]
            # memset can't target f32r; zero the ones-column via a uint32
            # view, then produce rounded-f32r 1.0s with ACT (0*x + 1).
            for j in range(J):
                ones_col = vt_sb[j][:, :, D:D + 1]
                nc.vector.memset(ones_col.bitcast(mybir.dt.uint32), 0)
                nc.scalar.activation(
                    out=ones_col, in_=ones_col,
                    func=mybir.ActivationFunctionType.Identity,
                    bias=1.0, scale=0.0,
                )
            for j in range(J):
                tag = tags[ti % 3]; ti += 1
                psv = pp.tile([P, 512], F32, tag=tag, name=tag,
                              bufs=2 if tag == "po" else 1)
                for kc in range(KC):
                    nc.tensor.matmul(
                        psv[:, 0:2 * P],
                        lhsT=x_sb[kc][:, j * P:(j + 1) * P],
                        rhs=wv_sb[kc],
                        start=(kc == 0),
                        stop=(kc == KC - 1),
                    )
                nc.vector.tensor_copy(
                    out=vt_sb[j][:, :, 0:D],
                    in_=psv[:, 0:2 * P].rearrange("p (h d) -> p h d", h=HL),
                )

            ones64 = consts.tile([P, D], F32R)
            nc.vector.memset(ones64.bitcast(mybir.dt.uint32), 0)
            nc.scalar.activation(
                out=ones64[D:D + 1, :], in_=ones64[D:D + 1, :],
                func=mybir.ActivationFunctionType.Identity,
                bias=1.0, scale=0.0,
            )

            # ---- attention: head pairs on alternating PE row groups,
            # i processed in halves; outproj per i-half overlaps the next
            # half's attention.
            y_sb = [
                consts.tile([D, NSEQ], F32R, tag=f"y{h}", name=f"y{h}")
                for h in range(HL)
            ]
            out_t = out.rearrange("(m p) n -> p m n", p=P)
            pending = []

            def emit_pending():
                for fn in pending:
                    fn()
                pending.clear()

            for ihalf in range(2):
                isl = slice(ihalf * 512, (ihalf + 1) * 512)
                for pair in ((0, 1) if ihalf == 0 else (1, 0)):
                    py = [
                        pp.tile([P, 512], F32, tag="py0", name="py0", bufs=1),
                        pp.tile([P, 512], F32, tag="py1", name="py1", bufs=1),
                    ]
                    for j in range(J):
                        # both lanes' scores into one 2-bank psum tile so a
                        # single 1024-wide exp serves the head pair
                        ps = pp.tile([P, 1024], F32, tag="s", name="s",
                                     bufs=2)
                        for lane in range(2):
                            hp = lane * D
                            nc.tensor.matmul(
                                ps[:, lane * 512:(lane + 1) * 512],
                                lhsT=k_sb[pair][hp:hp + D, j * P:(j + 1) * P],
                                rhs=q_sb[pair][hp:hp + D, isl],
                                start=True, stop=True,
                            )
                        e = epool.tile([P, 1024], F32R, tag="e", name="e")
                        nc.scalar.activation(
                            out=e, in_=ps,
                            func=mybir.ActivationFunctionType.Exp,
                        )
                        for lane in range(2):
                            nc.tensor.matmul(
                                py[lane][0:D + 1, :],
                                lhsT=vt_sb[j][:, 2 * pair + lane, :],
                                rhs=e[:, lane * 512:(lane + 1) * 512],
                                start=(j == 0), stop=(j == J - 1),
                            )
                    # the 65-row copies release the PSUM accumulators now;
                    # the replicate/1-over-x/scale tail is emitted one block
                    # later so its PE work lands in scheduling slack instead
                    # of pinching the next block's first scores.
                    emit_pending()
                    yus = []
                    for lane in range(2):
                        yu = ypool.tile([D + 1, 512], F32R, tag="yu",
                                        name="yu")
                        nc.vector.tensor_copy(out=yu, in_=py[lane][0:D + 1, :])
                        yus.append(yu)

                    def drain(yus=yus, pair=pair, isl=isl):
                        for lane in range(2):
                            h = 2 * pair + lane
                            pr = pp.tile([D, 512], F32, tag=f"py{lane}",
                                         name="pr", bufs=1)
                            nc.tensor.matmul(
                                pr, lhsT=ones64[D:D + 1, :],
                                rhs=yus[lane][D:D + 1, :],
                                start=True, stop=True,
                            )
                            rr = rpool.tile([D, 512], F32, tag="rr",
                                            name="rr")
                            nc.vector.reciprocal_approx_fast(out=rr, in_=pr)
                            nc.vector.tensor_tensor(
                                out=y_sb[h][:, isl],
                                in0=yus[lane][0:D, :], in1=rr,
                                op=mybir.AluOpType.mult,
                            )

                    drain()
                emit_pending()
                # output projection for this i-half (hoists into the next
                # half's attention once the 4 y tiles are ready)
                horder = (0, 1, 2, 3) if ihalf == 0 else (2, 3, 0, 1)
                for m in range(4):
                    po = pp.tile([P, 512], F32, tag="po", name="po", bufs=2)
                    for hi, h in enumerate(horder):
                        nc.tensor.matmul(
                            po,
                            lhsT=wot_sb[:, h, m * P:(m + 1) * P],
                            rhs=y_sb[h][:, isl],
                            start=(hi == 0),
                            stop=(hi == HL - 1),
                        )
                    ot = opool.tile([P, 512], F32, tag="ot")
                    nc.vector.tensor_copy(out=ot, in_=po)
                    nc.sync.dma_start(out=out_t[:, m, isl], in_=ot)

    nc.compile()
    return nc


def get_nc():
    if "nc" not in _NC_CACHE:
        _NC_CACHE["nc"] = build_nc()
    return _NC_CACHE["nc"]


def make_in_maps(x, Wq, Wk, Wv, Wo):
    in_maps = []
    for core in range(8):
        b, g = core // 2, core % 2
        sl = slice(g * 256, (g + 1) * 256)
        wqkv = np.stack(
            [Wq[sl, :].T, Wk[sl, :].T, Wv[sl, :].T], axis=1
        )  # (512, 3, 256)
        in_maps.append({
            "x": np.ascontiguousarray(x[b]),
            "wqkv_t": np.ascontiguousarray(wqkv),
            # [d, h, o] so lhsT slices are contiguous per head
            "wo_t": np.ascontiguousarray(
                Wo[:, sl].reshape(C, HL, D).transpose(2, 1, 0)
            ),
        })
    return in_maps


LAST_RESULTS = {}


def kernel(x, Wq, Wk, Wv, Wo, _trace=False):
    x = np.asarray(x, dtype=np.float32)
    Wq = np.asarray(Wq, dtype=np.float32)
    Wk = np.asarray(Wk, dtype=np.float32)
    Wv = np.asarray(Wv, dtype=np.float32)
    Wo = np.asarray(Wo, dtype=np.float32)

    nc = get_nc()
    in_maps = make_in_maps(x, Wq, Wk, Wv, Wo)
    res = run_bass_kernel_spmd(
        nc, in_maps, core_ids=list(range(8)), trace=_trace
    )
    LAST_RESULTS["res"] = res
    parts = [np.asarray(r["out_p"]) for r in res.results]
    out = np.stack([parts[2 * b] + parts[2 * b + 1] for b in range(4)])
    return out


# revision 35
# speedup vs baseline: 1.1097x; 1.0140x over previous
"""Multi-head attention Trainium2 Bass kernel.

Problem: x:(4,512,1024), Wq/Wk/Wv/Wo:(512,512), H=8 heads, d=64.
  q = Wq@x ; k = Wk@x ; v = Wv@x  (per batch, 1x1 conv == channel matmul)
  per head: S[i,j] = q[:,i].k[:,j] ; attn = softmax_j(S) ; y = attn @ v
  out = Wo @ y

Sharding: 8 cores = (batch b, head-half g).  Core (b,g) handles batch b,
local heads g*4..g*4+3 and computes the partial output
out_p = Wo[:, g*256:(g+1)*256] @ y_g which the host sums pairwise.

Layout: scores are computed TRANSPOSED (S^T[j,i] = k^T q) so no PE
transposes are needed anywhere; softmax runs without max subtraction
(scores max ~52 < 88 overflow limit); the PV matmul's lhsT carries an
extra ones column so the softmax denominator falls out of the same
matmul; the rowsum row is replicated across partitions by a K=1
ones-row matmul and inverted with a fast approximate reciprocal.

All matmuls are float32r: 1 row/cycle at 2.4 GHz once the PE clock gate
(HAM) is warm, with fp32-level dynamic range and ~1e-3 accuracy.
"""

import numpy as np

import concourse.bass as bass
import concourse.tile as tile
from concourse import bacc
from concourse import mybir
from concourse.bass_utils import run_bass_kernel_spmd

F32 = mybir.dt.float32
F32R = mybir.dt.float32r
BF16 = mybir.dt.bfloat16

P = 128
C = 512          # channels
NSEQ = 1024      # sequence length
D = 64           # head dim
HL = 4           # local heads per core
KC = C // P      # 4 contraction tiles over channels
J = NSEQ // P    # 8 key tiles

_NC_CACHE = {}


def build_nc():
    nc = bacc.Bacc("TRN2")

    x = nc.dram_tensor("x", [C, NSEQ], F32R, kind="ExternalInput")
    wqkv = nc.dram_tensor("wqkv_t", [C, 3, 2 * P], F32R, kind="ExternalInput")
    wo = nc.dram_tensor("wo_t", [D, HL, C], F32R, kind="ExternalInput")
    out = nc.dram_tensor("out_p", [C, NSEQ], F32, kind="ExternalOutput")

    with tile.TileContext(nc) as tc:
        with (
            tc.tile_pool(name="consts", bufs=1) as consts,
            tc.tile_pool(name="epool", bufs=6) as epool,
            tc.tile_pool(name="ypool", bufs=6) as ypool,
            tc.tile_pool(name="rpool", bufs=4) as rpool,
            tc.tile_pool(name="opool", bufs=2) as opool,
            tc.tile_pool(name="pp", bufs=2, space="PSUM") as pp,
        ):
            # ---- load inputs: per-k-tile tiles so Tile's per-tile dep
            # tracking lets each projection matmul start as soon as ITS
            # chunk is in, not after the whole load.
            x_t = x.rearrange("(kc p) n -> p kc n", p=P)
            w_t = wqkv.rearrange("(kc p) w m -> p kc w m", p=P)
            x_sb, w3_sb = [], []
            for kc in range(KC):
                t = consts.tile([P, 3, 2 * P], F32R, tag=f"w{kc}")
                nc.sync.dma_start(t, w_t[:, kc])
                w3_sb.append(t)
                xt = consts.tile([P, NSEQ], F32R, tag=f"x{kc}")
                nc.sync.dma_start(xt, x_t[:, kc])
                x_sb.append(xt)
            wq_sb = [t[:, 0, :] for t in w3_sb]
            wk_sb = [t[:, 1, :] for t in w3_sb]
            wv_sb = [t[:, 2, :] for t in w3_sb]
            # dependency-free warm-up matmuls fill the PE during the load
            # window so the HAM clock gate reaches 8/8 before projections.
            warm_sb = consts.tile([P, 512], F32R)
            nc.vector.memset(warm_sb.bitcast(mybir.dt.uint32), 0)
            for wi in range(14):
                pw = pp.tile([P, 512], F32, tag="po", name="pw", bufs=2)
                nc.tensor.matmul(pw, lhsT=warm_sb[:, 0:P], rhs=warm_sb,
                                 start=True, stop=True)
            wot_sb = consts.tile([D, HL, C], F32R)
            nc.sync.dma_start(wot_sb, wo[:, :, :])

            # ---- q, k projections: (256,512)@(512,1024) ----
            q_sb = [consts.tile([P, NSEQ], F32R, tag=f"q{m}", name=f"q{m}")
                    for m in range(2)]
            k_sb = [consts.tile([P, NSEQ], F32R, tag=f"k{m}", name=f"k{m}")
                    for m in range(2)]
            tags = ["po", "py0", "py1"]
            ti = 0
            for w_sb, dst in ((wq_sb, q_sb), (wk_sb, k_sb)):
                for m in range(2):
                    for nn in range(2):
                        tag = tags[ti % 3]; ti += 1
                        ps = pp.tile([P, 512], F32, tag=tag, name=tag,
                                     bufs=2 if tag == "po" else 1)
                        for kc in range(KC):
                            nc.tensor.matmul(
                                ps,
                                lhsT=w_sb[kc][:, m * P:(m + 1) * P],
                                rhs=x_sb[kc][:, nn * 512:(nn + 1) * 512],
                                start=(kc == 0),
                                stop=(kc == KC - 1),
                            )
                        nc.vector.tensor_copy(
                            out=dst[m][:, nn * 512:(nn + 1) * 512], in_=ps
                        )

            # ---- v^T projection: out[j, d'] tiles, plus ones column ----
            vt_sb = [
                consts.tile([P, HL, D + 1], F32R, tag=f"vt{j}", name=f"vt{j}")
                for j in range(J)
            ]
            # memset can't target f32r; zero the ones-column via a uint32
            # view, then produce rounded-f32r 1.0s with ACT (0*x + 1).
            for j in range(J):
                ones_col = vt_sb[j][:, :, D:D + 1]
                nc.vector.memset(ones_col.bitcast(mybir.dt.uint32), 0)
                nc.scalar.activation(
                    out=ones_col, in_=ones_col,
                    func=mybir.ActivationFunctionType.Identity,
                    bias=1.0, scale=0.0,
                )
            for j in range(J):
                tag = tags[ti % 3]; ti += 1
                psv = pp.tile([P, 512], F32, tag=tag, name=tag,
                              bufs=2 if tag == "po" else 1)
                for kc in range(KC):
                    nc.tensor.matmul(
                        psv[:, 0:2 * P],
                        lhsT=x_sb[kc][:, j * P:(j + 1) * P],
                        rhs=wv_sb[kc],
                        start=(kc == 0),
                        stop=(kc == KC - 1),
                    )
                nc.vector.tensor_copy(
                    out=vt_sb[j][:, :, 0:D],
                    in_=psv[:, 0:2 * P].rearrange("p (h d) -> p h d", h=HL),
                )

            ones64 = consts.tile([P, D], F32R)
            nc.vector.memset(ones64.bitcast(mybir.dt.uint32), 0)
            nc.scalar.activation(
                out=ones64[D:D + 1, :], in_=ones64[D:D + 1, :],
                func=mybir.ActivationFunctionType.Identity,
                bias=1.0, scale=0.0,
            )

            # ---- attention: head pairs on alternating PE row groups,
            # i processed in halves; outproj per i-half overlaps the next
            # half's attention.
            y_sb = [
                consts.tile([D, NSEQ], F32R, tag=f"y{h}", name=f"y{h}")
                for h in range(HL)
            ]
            out_t = out.rearrange("(m p) n -> p m n", p=P)
            pending = []

            def emit_pending():
                for fn in pending:
                    fn()
                pending.clear()

            for ihalf in range(2):
                isl = slice(ihalf * 512, (ihalf + 1) * 512)
                for pair in ((0, 1) if ihalf == 0 else (1, 0)):
                    py = [
                        pp.tile([P, 512], F32, tag="py0", name="py0", bufs=1),
                        pp.tile([P, 512], F32, tag="py1", name="py1", bufs=1),
                    ]
                    for j in range(J):
                        # both lanes' scores into one 2-bank psum tile so a
                        # single 1024-wide exp serves the head pair
                        ps = pp.tile([P, 1024], F32, tag="s", name="s",
                                     bufs=2)
                        for lane in range(2):
                            hp = lane * D
                            nc.tensor.matmul(
                                ps[:, lane * 512:(lane + 1) * 512],
                                lhsT=k_sb[pair][hp:hp + D, j * P:(j + 1) * P],
                                rhs=q_sb[pair][hp:hp + D, isl],
                                start=True, stop=True,
                            )
                        e = epool.tile([P, 1024], F32R, tag="e", name="e")
                        nc.scalar.activation(
                            out=e, in_=ps,
                            func=mybir.ActivationFunctionType.Exp,
                        )
                        for lane in range(2):
                            nc.tensor.matmul(
                                py[lane][0:D + 1, :],
                                lhsT=vt_sb[j][:, 2 * pair + lane, :],
                                rhs=e[:, lane * 512:(lane + 1) * 512],
                                start=(j == 0), stop=(j == J - 1),
                            )
                    # the 65-row copies release the PSUM accumulators now;
                    # the replicate/1-over-x/scale tail is emitted one block
                    # later so its PE work lands in scheduling slack instead
                    # of pinching the next block's first scores.
                    emit_pending()
                    yus = []
                    for lane in range(2):
                        yu = ypool.tile([D + 1, 512], F32R, tag="yu",
                                        name="yu")
                        nc.vector.tensor_copy(out=yu, in_=py[lane][0:D + 1, :])
                        yus.append(yu)

                    def drain(yus=yus, pair=pair, isl=isl):
                        for lane in range(2):
                            h = 2 * pair + lane
                            pr = pp.tile([D, 512], F32, tag=f"py{lane}",
                                         name="pr", bufs=1)
                            nc.tensor.matmul(
                                pr, lhsT=ones64[D:D + 1, :],
                                rhs=yus[lane][D:D + 1, :],
                                start=True, stop=True,
                            )
                            rr = rpool.tile([D, 512], F32, tag="rr",
                                            name="rr")
                            nc.vector.reciprocal_approx_fast(out=rr, in_=pr)
                            nc.vector.tensor_tensor(
                                out=y_sb[h][:, isl],
                                in0=yus[lane][0:D, :], in1=rr,
                                op=mybir.AluOpType.mult,
                            )

                    drain()
                emit_pending()
                # output projection for this i-half (hoists into the next
                # half's attention once the 4 y tiles are ready)
                horder = (0, 1, 2, 3) if ihalf == 0 else (2, 3, 0, 1)
                for m in range(4):
                    po = pp.tile([P, 512], F32, tag="po", name="po", bufs=2)
                    for hi, h in enumerate(horder):
                        nc.tensor.matmul(
                            po,
                            lhsT=wot_sb[:, h, m * P:(m + 1) * P],
                            rhs=y_sb[h][:, isl],
                            start=(hi == 0),
                            stop=(hi == HL - 1),
                        )
                    ot = opool.tile([P, 512], F32, tag="ot")
                    nc.vector.tensor_copy(out=ot, in_=po)
                    nc.sync.dma_start(out=out_t[:, m, isl], in_=ot)

    nc.compile()
    return nc


def get_nc():
    if "nc" not in _NC_CACHE:
        _NC_CACHE["nc"] = build_nc()
    return _NC_CACHE["nc"]


def make_in_maps(x, Wq, Wk, Wv, Wo):
    in_maps = []
    for core in range(8):
        b, g = core // 2, core % 2
        sl = slice(g * 256, (g + 1) * 256)
        wqkv = np.stack(
            [Wq[sl, :].T, Wk[sl, :].T, Wv[sl, :].T], axis=1
        )  # (512, 3, 256)
        in_maps.append({
            "x": np.ascontiguousarray(x[b]),
            "wqkv_t": np.ascontiguousarray(wqkv),
            # [d, h, o] so lhsT slices are contiguous per head
            "wo_t": np.ascontiguousarray(
                Wo[:, sl].reshape(C, HL, D).transpose(2, 1, 0)
            ),
        })
    return in_maps


LAST_RESULTS = {}


def kernel(x, Wq, Wk, Wv, Wo, _trace=False):
    x = np.asarray(x, dtype=np.float32)
    Wq = np.asarray(Wq, dtype=np.float32)
    Wk = np.asarray(Wk, dtype=np.float32)
    Wv = np.asarray(Wv, dtype=np.float32)
    Wo = np.asarray(Wo, dtype=np.float32)

    nc = get_nc()
    in_maps = make_in_maps(x, Wq, Wk, Wv, Wo)
    res = run_bass_kernel_spmd(
        nc, in_maps, core_ids=list(range(8)), trace=_trace
    )
    LAST_RESULTS["res"] = res
    parts = [np.asarray(r["out_p"]) for r in res.results]
    out = np.stack([parts[2 * b] + parts[2 * b + 1] for b in range(4)])
    return out
